# revision 42
# baseline (speedup 1.0000x reference)
"""Trainium2 Bass kernel for nn_BlockRF (BatchNorm -> LocallyConnected2D 3x3 valid -> ReLU).

Shapes (hardcoded per the problem spec):
  x:      [B=32, H=64, W=64, C=32]  f32
  gamma/beta/moving_mean/moving_var: [C=32] f32
  weight: [OH*OW=3844, KH*KW*C=288, F=32] f32
  out:    [B=32, OH=62, OW=62, F=32] f32

v2 design (memory regime: the weight tensor dominates traffic):
  - Shard over output rows: 8 rows/core on 8 cores (OH padded 62->64),
    processed as 4 row PAIRS per core.
  - Weights shipped as fp8 E3M4 (x32 exponent centering, folded back into the
    BN affine on the x side) -> halves the dominant HBM stream. Mixed-dtype
    matmul: fp16 stationary x fp8 moving (empirical rel-err ~1.3e-2 vs the
    2e-2 gate).
  - One x stationary [128,32] (rows oh..oh+3 x 32ch) serves BOTH rows of a
    pair: even row matmuls take K=96 (rows 0..95), odd row K=128 against a
    weight tile whose rows 0..31 are zero (host-padded into the odd DMA).
  - PE col groups: group b = tile_position (0,32b) owns position bank b
    (16 positions); each output row's PSUM is a single [128,512] bank
    (4 groups x disjoint 32-partition blocks), 8 banks in flight.
  - Weight streams are split into wave halves A (w-offsets 0..8 per group)
    and B (9..17): matmuls of a half start as soon as that half's ~0.3MB
    transfer lands. A-halves ride the sync HWDGE ring, B-halves the scalar
    ring, x + y the gpsimd SWDGE ring. 3-deep weight / 4-deep x buffering
    keeps every DMA queue busy without write-after-read stalls.
  - memset+accumulate PSUM discipline; ReLU evacuation as one [128,512] op
    per row (Vector for even rows, Scalar for odd); one [128,2KB] output
    DMA per pair.

Host side only pads/transposes/casts (layout prep + sharding) - all model
arithmetic (BN, conv, ReLU) runs on device.
"""

import numpy as np
import ml_dtypes

B, H, W, C, F = 32, 64, 64, 32, 32
KH = KW = 3
OH = OW = 62
OHP = OWP = 64
RPC = 8          # output rows per core
NPAIR = RPC // 2
XFREE = W * B    # 2048
EPS = 1e-3
WSCALE = 32.0    # weight exponent centering for fp8 E3M4
NGRP = 4         # PE col groups == position banks of 16
TSPLITS = [0, 9, 18]  # wave ranges of the weight-stream chunks


def _build_sched():
    """Static MM schedule. Group b (PE col-group b) owns positions
    16b..16b+15 (its own single-bank PSUM region at partitions 32b). Its
    w-list covers w=16b+t: t=0..15 main (position subrange clipped to the
    bank) and t=16,17 boundary work for positions 14,15 of its bank coming
    from w in the next group's range.

    The schedule is cut into len(TSPLITS)-1 wave-range chunks; each chunk's
    slots are packed contiguously so its weights ship as one transfer and
    its matmuls start as soon as that transfer lands (and the PE never
    idles long enough for the HAM clock gate to re-throttle).

    Returns a list of (issue, nslot) per chunk; issue entries are
    (b, w, plo, phi, slot) with plo/phi bank-local, slot chunk-local."""
    pergroup = {}
    for b in range(NGRP):
        tmax = 17 if b < NGRP - 1 else 15
        for t in range(tmax + 1):
            w = 16 * b + t
            if t == 0:
                lo, hi = 16 * b, 16 * b
            elif t == 1:
                lo, hi = 16 * b, 16 * b + 1
            elif t <= 15:
                lo, hi = 16 * b + t - 2, 16 * b + t
            elif t == 16:
                lo, hi = 16 * b + 14, 16 * b + 15
            else:
                lo, hi = 16 * b + 15, 16 * b + 15
            pergroup[(b, t)] = (w, lo, hi)
    chunks = []
    for ci in range(len(TSPLITS) - 1):
        issue = []
        off = 0
        for t in range(TSPLITS[ci], TSPLITS[ci + 1]):
            for b in range(NGRP):
                if (b, t) not in pergroup:
                    continue
                w, lo, hi = pergroup[(b, t)]
                issue.append((b, w, lo - 16 * b, hi - 16 * b, off))
                off += hi - lo + 1
        chunks.append((issue, off))
    return chunks


CHUNKS = _build_sched()
NCHUNK = len(CHUNKS)
SLOTF_K = [n * F for (_, n) in CHUNKS]

_CACHE = {}


def _build_program():
    import concourse.mybir as mybir
    import concourse.tile as tile
    from concourse import bacc
    from contextlib import ExitStack

    f16 = mybir.dt.float16
    f32 = mybir.dt.float32
    f8 = mybir.dt.float8e3

    nc = bacc.Bacc("TRN2", target_bir_lowering=False, debug=False, num_devices=8)

    xin = nc.dram_tensor("xin", [NPAIR, 128, XFREE], f16, kind="ExternalInput").ap()
    weink = [nc.dram_tensor(f"wein{k}", [NPAIR, 96, SLOTF_K[k]], f8,
                            kind="ExternalInput").ap() for k in range(NCHUNK)]
    woink = [nc.dram_tensor(f"woin{k}", [NPAIR, 128, SLOTF_K[k]], f8,
                            kind="ExternalInput").ap() for k in range(NCHUNK)]
    pin = nc.dram_tensor("pin", [128, 2], f32, kind="ExternalInput").ap()
    yout = nc.dram_tensor("yout", [NPAIR, 128, 1024], f16, kind="ExternalOutput").ap()

    with ExitStack() as ctx:
        tc = ctx.enter_context(tile.TileContext(nc))
        singles = ctx.enter_context(tc.tile_pool(name="singles", bufs=1))
        xpool = ctx.enter_context(tc.tile_pool(name="xpool", bufs=4))
        xnpool = ctx.enter_context(tc.tile_pool(name="xnpool", bufs=4))
        wepools = [ctx.enter_context(tc.tile_pool(name=f"wep{k}", bufs=3))
                   for k in range(NCHUNK)]
        wopools = [ctx.enter_context(tc.tile_pool(name=f"wop{k}", bufs=3))
                   for k in range(NCHUNK)]
        ypool = ctx.enter_context(tc.tile_pool(name="ypool", bufs=2))
        pspool = ctx.enter_context(tc.tile_pool(name="pspool", bufs=8, space="PSUM"))

        xts = []

        def fetch_x(p):
            xt = xpool.tile([128, XFREE], f16, name="xt", tag="xt")
            nc.gpsimd.dma_start(out=xt, in_=xin[p])
            xts.append(xt)

        # BN coefficients (1KB) lead the sync ring; x leads the gpsimd ring;
        # the two HWDGE rings otherwise carry the weight streams, balanced
        # per pair (even-A + odd-B on sync, odd-A + even-B on scalar)
        par = singles.tile([128, 2], f32)
        nc.sync.dma_start(out=par, in_=pin)
        for p in range(NPAIR):
            fetch_x(p)

        wek = {}
        wok = {}

        def fetch_weights(p):
            for k in range(NCHUNK):
                we = wepools[k].tile([96, SLOTF_K[k]], f8, name=f"we{k}",
                                     tag=f"we{k}")
                wo = wopools[k].tile([128, SLOTF_K[k]], f8, name=f"wo{k}",
                                     tag=f"wo{k}")
                if k % 2 == 0:
                    nc.sync.dma_start(out=we, in_=weink[k][p])
                    nc.scalar.dma_start(out=wo, in_=woink[k][p])
                else:
                    nc.scalar.dma_start(out=we, in_=weink[k][p])
                    nc.sync.dma_start(out=wo, in_=woink[k][p])
                wek[(p, k)] = we
                wok[(p, k)] = wo

        fetch_weights(0)

        def psum_alloc():
            ps = pspool.tile([128, 512], f32, name="ps", tag="ps")
            nc.vector.memset(ps, 0.0)
            return ps

        # BN affine coefficients are folded on the host (standard inference
        # BN fold): par[:,0] = gamma/sqrt(var+eps)/WSCALE, par[:,1] = (beta -
        # mean*gamma/sqrt(var+eps))/WSCALE. The per-activation normalize
        # itself runs here on the Vector engine.
        def bn(p):
            xn = xnpool.tile([128, XFREE], f16, name="xn", tag="xn")
            nc.vector.tensor_scalar(
                xn, xts[p], par[:, 0:1], par[:, 1:2],
                op0=mybir.AluOpType.mult, op1=mybir.AluOpType.add,
            )
            return xn

        # all BNs + PSUM memsets up front on the vector queue (8 PSUM banks
        # = exactly 4 pairs, so nothing recycles and nothing queues behind
        # an evacuation)
        xns, pss = {}, {}
        for p in range(NPAIR):
            xns[p] = bn(p)
            pss[p] = (psum_alloc(), psum_alloc())

        def emit_half(issue, xn, pse, pso, wet, wot):
            for (b, w, plo, phi, s) in issue:
                n = phi - plo + 1
                ocol = slice(plo * F, (phi + 1) * F)
                wcol = slice(s * F, (s + n) * F)
                xcol = slice(w * B, (w + 1) * B)
                nc.tensor.matmul(
                    pso[32 * b:32 * b + 32, ocol],
                    xn[0:128, xcol],
                    wot[0:128, wcol],
                    start=False, stop=True,
                    skip_group_check=True,
                    tile_position=(0, 32 * b),
                )
                nc.tensor.matmul(
                    pse[32 * b:32 * b + 32, ocol],
                    xn[0:96, xcol],
                    wet[0:96, wcol],
                    start=False, stop=True,
                    skip_group_check=True,
                    tile_position=(0, 32 * b),
                )

        fetch_weights(1)
        fetch_weights(2)

        for p in range(NPAIR):
            if p + 3 < NPAIR:
                fetch_weights(p + 3)

            xn = xns[p]
            pse, pso = pss[p]
            for k in range(NCHUNK):
                emit_half(CHUNKS[k][0], xn, pse, pso,
                          wek[(p, k)], wok[(p, k)])

            # evacuate + store each row as soon as its PSUM closes; y0-2 ride
            # the (by then idle) gpsimd ring, y3 takes the two HWDGE rings
            # for their lower dispatch latency on the kernel tail
            yb = ypool.tile([128, 1024], f16, name="yb", tag="yb")
            nc.vector.tensor_scalar_max(yb[:, 0:512], pse, 0.0)
            if p == NPAIR - 1:
                nc.sync.dma_start(out=yout[p][:, 0:512], in_=yb[:, 0:512])
            else:
                nc.gpsimd.dma_start(out=yout[p][:, 0:512], in_=yb[:, 0:512])
            nc.vector.tensor_scalar_max(yb[:, 512:1024], pso, 0.0)
            if p == NPAIR - 1:
                nc.scalar.dma_start(out=yout[p][:, 512:1024],
                                    in_=yb[:, 512:1024])
            else:
                nc.gpsimd.dma_start(out=yout[p][:, 512:1024],
                                    in_=yb[:, 512:1024])

    nc.compile()
    return nc


def _get_program():
    if "nc" not in _CACHE:
        _CACHE["nc"] = _build_program()
    return _CACHE["nc"]


def _prep_inputs(x, gamma, beta, moving_mean, moving_var, weight):
    """Host-side shard/layout/cast prep. Returns per-core in_maps."""
    x = np.asarray(x, dtype=np.float32)
    weight = np.asarray(weight, dtype=np.float32)

    # x: [B,H,W,C] -> pad H to 66 -> [h, c, w, b] fp16
    xpad = np.zeros((B, H + 2, W, C), np.float32)
    xpad[:, :H] = x
    xt_all = np.ascontiguousarray(xpad.transpose(1, 3, 2, 0)).astype(np.float16)

    # weights -> wave-major slot layout per stream chunk, fp8 E3M4, x WSCALE
    w6 = weight.reshape(OH, OW, KH, KW, C, F) * WSCALE
    wpad = np.zeros((OHP, OWP, KH, KW, C, F), np.float32)
    wpad[:OH, :OW] = w6
    wks = []
    for (issue, nslot) in CHUNKS:
        dst = np.zeros((OHP, 96, nslot, F), np.float32)
        for (b, w, plo, phi, s) in issue:
            for k, pl in enumerate(range(plo, phi + 1)):
                pos = 16 * b + pl
                j = w - pos
                dst[:, :, s + k, :] = wpad[:, pos, :, j, :, :].reshape(
                    OHP, 96, F)
        wks.append(dst.astype(ml_dtypes.float8_e3m4))

    inv = (gamma / np.sqrt(moving_var + EPS)).astype(np.float32)
    acoef = inv / WSCALE
    bcoef = (beta - moving_mean * inv) / WSCALE
    p128 = np.tile(
        np.stack([acoef, bcoef], axis=1).astype(np.float32), (4, 1)
    )  # [128, 2]

    in_maps = []
    for k in range(8):
        R = k * RPC
        xc = np.stack(
            [xt_all[R + 2 * p: R + 2 * p + 4].reshape(128, XFREE)
             for p in range(NPAIR)]
        )  # [NPAIR, 128, 2048]
        im = {"xin": xc, "pin": p128}
        for ci, wk in enumerate(wks):
            sf = SLOTF_K[ci]
            im[f"wein{ci}"] = np.ascontiguousarray(
                wk[R + 0: R + RPC: 2]).reshape(NPAIR, 96, sf)
            # odd rows: zero band at partitions 0..31 (K=128 matmul)
            wo = np.zeros((NPAIR, 128, sf), ml_dtypes.float8_e3m4)
            wo[:, 32:] = wk[R + 1: R + RPC: 2].reshape(NPAIR, 96, sf)
            im[f"woin{ci}"] = wo
        in_maps.append(im)
    return in_maps


def _assemble_output(results):
    """results: per-core {"yout": [NPAIR, 128, 1024] f16} -> [B, OH, OW, F] f32."""
    ys = []
    for r in results:
        yd = np.asarray(r["yout"]).astype(np.float32)
        y6 = yd.reshape(4, 4, 32, 2, 16, 32)   # [pair, bank, batch, ohp, posin, f]
        y6 = y6.transpose(2, 0, 3, 1, 4, 5)     # [batch, pair, ohp, bank, posin, f]
        ys.append(y6.reshape(32, 8, 64, 32))
    y = np.concatenate(ys, axis=1)              # [B, 8*ncores, OWP, F]
    return np.ascontiguousarray(y[:, :min(OH, y.shape[1]), :OW, :])


def run(inputs, trace=False, trace_cores=None):
    """Build/compile/run on 8 cores. Returns (y, BassKernelResults)."""
    from concourse.bass_utils import run_bass_kernel_spmd

    nc = _get_program()
    in_maps = _prep_inputs(**inputs)
    res = run_bass_kernel_spmd(
        nc,
        in_maps,
        core_ids=list(range(8)),
        trace=trace,
        **({"trace_cores": trace_cores} if trace_cores is not None else {}),
    )
    return _assemble_output(res.results), res


def kernel(x, gamma, beta, moving_mean, moving_var, weight):
    y, _ = run(
        dict(x=x, gamma=gamma, beta=beta, moving_mean=moving_mean,
             moving_var=moving_var, weight=weight)
    )
    return y


# revision 43
# speedup vs baseline: 1.0894x; 1.0894x over previous
"""Trainium2 Bass kernel for nn_BlockRF (BatchNorm -> LocallyConnected2D 3x3 valid -> ReLU).

Shapes (hardcoded per the problem spec):
  x:      [B=32, H=64, W=64, C=32]  f32
  gamma/beta/moving_mean/moving_var: [C=32] f32
  weight: [OH*OW=3844, KH*KW*C=288, F=32] f32
  out:    [B=32, OH=62, OW=62, F=32] f32

v2 design (memory regime: the weight tensor dominates traffic):
  - Shard over output rows: 8 rows/core on 8 cores (OH padded 62->64),
    processed as 4 row PAIRS per core.
  - Weights shipped as fp8 E3M4 (x32 exponent centering, folded back into the
    BN affine on the x side) -> halves the dominant HBM stream. Mixed-dtype
    matmul: fp16 stationary x fp8 moving (empirical rel-err ~1.3e-2 vs the
    2e-2 gate).
  - One x stationary [128,32] (rows oh..oh+3 x 32ch) serves BOTH rows of a
    pair: even row matmuls take K=96 (rows 0..95), odd row K=128 against a
    weight tile whose rows 0..31 are zero (host-padded into the odd DMA).
  - PE col groups: group b = tile_position (0,32b) owns position bank b
    (16 positions); each output row's PSUM is a single [128,512] bank
    (4 groups x disjoint 32-partition blocks), 8 banks in flight.
  - Weight streams are split into wave halves A (w-offsets 0..8 per group)
    and B (9..17): matmuls of a half start as soon as that half's ~0.3MB
    transfer lands. A-halves ride the sync HWDGE ring, B-halves the scalar
    ring, x + y the gpsimd SWDGE ring. 3-deep weight / 4-deep x buffering
    keeps every DMA queue busy without write-after-read stalls.
  - memset+accumulate PSUM discipline; ReLU evacuation as one [128,512] op
    per row (Vector for even rows, Scalar for odd); one [128,2KB] output
    DMA per pair.

Host side only pads/transposes/casts (layout prep + sharding) - all model
arithmetic (BN, conv, ReLU) runs on device.
"""

import numpy as np
import ml_dtypes

B, H, W, C, F = 32, 64, 64, 32, 32
KH = KW = 3
OH = OW = 62
OHP = OWP = 64
RPC = 8          # output rows per core
NPAIR = RPC // 2
XFREE = W * B    # 2048
EPS = 1e-3
WSCALE = 32.0    # weight exponent centering for fp8 E3M4
NGRP = 4         # PE col groups == position banks of 16
TSPLITS = [0, 9, 18]  # wave ranges of the weight-stream chunks


def _build_sched():
    """Static MM schedule. Group b (PE col-group b) owns positions
    16b..16b+15 (its own single-bank PSUM region at partitions 32b). Its
    w-list covers w=16b+t: t=0..15 main (position subrange clipped to the
    bank) and t=16,17 boundary work for positions 14,15 of its bank coming
    from w in the next group's range.

    The schedule is cut into len(TSPLITS)-1 wave-range chunks; each chunk's
    slots are packed contiguously so its weights ship as one transfer and
    its matmuls start as soon as that transfer lands (and the PE never
    idles long enough for the HAM clock gate to re-throttle).

    Returns a list of (issue, nslot) per chunk; issue entries are
    (b, w, plo, phi, slot) with plo/phi bank-local, slot chunk-local."""
    pergroup = {}
    for b in range(NGRP):
        tmax = 17 if b < NGRP - 1 else 15
        for t in range(tmax + 1):
            w = 16 * b + t
            if t == 0:
                lo, hi = 16 * b, 16 * b
            elif t == 1:
                lo, hi = 16 * b, 16 * b + 1
            elif t <= 15:
                lo, hi = 16 * b + t - 2, 16 * b + t
            elif t == 16:
                lo, hi = 16 * b + 14, 16 * b + 15
            else:
                lo, hi = 16 * b + 15, 16 * b + 15
            pergroup[(b, t)] = (w, lo, hi)
    chunks = []
    for ci in range(len(TSPLITS) - 1):
        issue = []
        off = 0
        for t in range(TSPLITS[ci], TSPLITS[ci + 1]):
            for b in range(NGRP):
                if (b, t) not in pergroup:
                    continue
                w, lo, hi = pergroup[(b, t)]
                issue.append((b, w, lo - 16 * b, hi - 16 * b, off))
                off += hi - lo + 1
        chunks.append((issue, off))
    return chunks


CHUNKS = _build_sched()
NCHUNK = len(CHUNKS)
SLOTF_K = [n * F for (_, n) in CHUNKS]

_CACHE = {}


def _build_program():
    import concourse.mybir as mybir
    import concourse.tile as tile
    from concourse import bacc
    from contextlib import ExitStack

    f16 = mybir.dt.float16
    f32 = mybir.dt.float32
    f8 = mybir.dt.float8e3

    nc = bacc.Bacc("TRN2", target_bir_lowering=False, debug=False, num_devices=8)

    xin = nc.dram_tensor("xin", [NPAIR, 128, XFREE], f16, kind="ExternalInput").ap()
    weink = [nc.dram_tensor(f"wein{k}", [NPAIR, 96, SLOTF_K[k]], f8,
                            kind="ExternalInput").ap() for k in range(NCHUNK)]
    woink = [nc.dram_tensor(f"woin{k}", [NPAIR, 128, SLOTF_K[k]], f8,
                            kind="ExternalInput").ap() for k in range(NCHUNK)]
    pin = nc.dram_tensor("pin", [128, 2], f32, kind="ExternalInput").ap()
    yout = nc.dram_tensor("yout", [NPAIR, 128, 1024], f16, kind="ExternalOutput").ap()

    with ExitStack() as ctx:
        tc = ctx.enter_context(tile.TileContext(nc))
        singles = ctx.enter_context(tc.tile_pool(name="singles", bufs=1))
        xpool = ctx.enter_context(tc.tile_pool(name="xpool", bufs=4))
        xnpool = ctx.enter_context(tc.tile_pool(name="xnpool", bufs=4))
        wepools = [ctx.enter_context(tc.tile_pool(name=f"wep{k}", bufs=3))
                   for k in range(NCHUNK)]
        wopools = [ctx.enter_context(tc.tile_pool(name=f"wop{k}", bufs=3))
                   for k in range(NCHUNK)]
        ypool = ctx.enter_context(tc.tile_pool(name="ypool", bufs=2))
        pspool = ctx.enter_context(tc.tile_pool(name="pspool", bufs=8, space="PSUM"))

        xts = []

        def fetch_x(p):
            xt = xpool.tile([128, XFREE], f16, name="xt", tag="xt")
            nc.gpsimd.dma_start(out=xt, in_=xin[p])
            xts.append(xt)

        # BN coefficients (1KB) lead the sync ring; x leads the gpsimd ring;
        # the two HWDGE rings otherwise carry the weight streams, balanced
        # per pair (even-A + odd-B on sync, odd-A + even-B on scalar)
        par = singles.tile([128, 2], f32)
        nc.sync.dma_start(out=par, in_=pin)
        for p in range(NPAIR):
            fetch_x(p)

        wek = {}
        wok = {}

        def fetch_weights(p):
            for k in range(NCHUNK):
                we = wepools[k].tile([96, SLOTF_K[k]], f8, name=f"we{k}",
                                     tag=f"we{k}")
                wo = wopools[k].tile([128, SLOTF_K[k]], f8, name=f"wo{k}",
                                     tag=f"wo{k}")
                if k % 2 == 0:
                    nc.sync.dma_start(out=we, in_=weink[k][p])
                    nc.scalar.dma_start(out=wo, in_=woink[k][p])
                else:
                    nc.scalar.dma_start(out=we, in_=weink[k][p])
                    nc.sync.dma_start(out=wo, in_=woink[k][p])
                wek[(p, k)] = we
                wok[(p, k)] = wo

        fetch_weights(0)

        def psum_alloc():
            ps = pspool.tile([128, 512], f32, name="ps", tag="ps")
            nc.vector.memset(ps, 0.0)
            return ps

        # BN affine coefficients are folded on the host (standard inference
        # BN fold): par[:,0] = gamma/sqrt(var+eps)/WSCALE, par[:,1] = (beta -
        # mean*gamma/sqrt(var+eps))/WSCALE. The per-activation normalize
        # itself runs here on the Vector engine.
        def bn(p):
            xn = xnpool.tile([128, XFREE], f16, name="xn", tag="xn")
            nc.vector.tensor_scalar(
                xn, xts[p], par[:, 0:1], par[:, 1:2],
                op0=mybir.AluOpType.mult, op1=mybir.AluOpType.add,
            )
            return xn

        # all BNs + PSUM memsets up front on the vector queue (8 PSUM banks
        # = exactly 4 pairs, so nothing recycles and nothing queues behind
        # an evacuation)
        xns, pss = {}, {}
        for p in range(NPAIR):
            xns[p] = bn(p)
            pss[p] = (psum_alloc(), psum_alloc())

        def emit_half(issue, xn, pse, pso, wet, wot):
            for (b, w, plo, phi, s) in issue:
                n = phi - plo + 1
                ocol = slice(plo * F, (phi + 1) * F)
                wcol = slice(s * F, (s + n) * F)
                xcol = slice(w * B, (w + 1) * B)
                nc.tensor.matmul(
                    pso[32 * b:32 * b + 32, ocol],
                    xn[0:128, xcol],
                    wot[0:128, wcol],
                    start=False, stop=True,
                    skip_group_check=True,
                    tile_position=(0, 32 * b),
                )
                nc.tensor.matmul(
                    pse[32 * b:32 * b + 32, ocol],
                    xn[0:96, xcol],
                    wet[0:96, wcol],
                    start=False, stop=True,
                    skip_group_check=True,
                    tile_position=(0, 32 * b),
                )

        fetch_weights(1)
        fetch_weights(2)

        for p in range(NPAIR):
            if p + 3 < NPAIR:
                fetch_weights(p + 3)

            xn = xns[p]
            pse, pso = pss[p]
            for k in range(NCHUNK):
                emit_half(CHUNKS[k][0], xn, pse, pso,
                          wek[(p, k)], wok[(p, k)])

            yb = ypool.tile([128, 1024], f16, name="yb", tag="yb")
            nc.vector.tensor_scalar_max(yb[:, 0:512], pse, 0.0)
            nc.vector.tensor_scalar_max(yb[:, 512:1024], pso, 0.0)
            # y0-2 ride the (by then idle) gpsimd ring; y3 takes the sync
            # HWDGE ring for its lower dispatch latency on the kernel tail
            if p == NPAIR - 1:
                nc.sync.dma_start(out=yout[p], in_=yb)
            else:
                nc.gpsimd.dma_start(out=yout[p], in_=yb)

    nc.compile()
    return nc


def _get_program():
    if "nc" not in _CACHE:
        _CACHE["nc"] = _build_program()
    return _CACHE["nc"]


def _prep_inputs(x, gamma, beta, moving_mean, moving_var, weight):
    """Host-side shard/layout/cast prep. Returns per-core in_maps."""
    x = np.asarray(x, dtype=np.float32)
    weight = np.asarray(weight, dtype=np.float32)

    # x: [B,H,W,C] -> pad H to 66 -> [h, c, w, b] fp16
    xpad = np.zeros((B, H + 2, W, C), np.float32)
    xpad[:, :H] = x
    xt_all = np.ascontiguousarray(xpad.transpose(1, 3, 2, 0)).astype(np.float16)

    # weights -> wave-major slot layout per stream chunk, fp8 E3M4, x WSCALE
    w6 = weight.reshape(OH, OW, KH, KW, C, F) * WSCALE
    wpad = np.zeros((OHP, OWP, KH, KW, C, F), np.float32)
    wpad[:OH, :OW] = w6
    wks = []
    for (issue, nslot) in CHUNKS:
        dst = np.zeros((OHP, 96, nslot, F), np.float32)
        for (b, w, plo, phi, s) in issue:
            for k, pl in enumerate(range(plo, phi + 1)):
                pos = 16 * b + pl
                j = w - pos
                dst[:, :, s + k, :] = wpad[:, pos, :, j, :, :].reshape(
                    OHP, 96, F)
        wks.append(dst.astype(ml_dtypes.float8_e3m4))

    inv = (gamma / np.sqrt(moving_var + EPS)).astype(np.float32)
    acoef = inv / WSCALE
    bcoef = (beta - moving_mean * inv) / WSCALE
    p128 = np.tile(
        np.stack([acoef, bcoef], axis=1).astype(np.float32), (4, 1)
    )  # [128, 2]

    in_maps = []
    for k in range(8):
        R = k * RPC
        xc = np.stack(
            [xt_all[R + 2 * p: R + 2 * p + 4].reshape(128, XFREE)
             for p in range(NPAIR)]
        )  # [NPAIR, 128, 2048]
        im = {"xin": xc, "pin": p128}
        for ci, wk in enumerate(wks):
            sf = SLOTF_K[ci]
            im[f"wein{ci}"] = np.ascontiguousarray(
                wk[R + 0: R + RPC: 2]).reshape(NPAIR, 96, sf)
            # odd rows: zero band at partitions 0..31 (K=128 matmul)
            wo = np.zeros((NPAIR, 128, sf), ml_dtypes.float8_e3m4)
            wo[:, 32:] = wk[R + 1: R + RPC: 2].reshape(NPAIR, 96, sf)
            im[f"woin{ci}"] = wo
        in_maps.append(im)
    return in_maps


def _assemble_output(results):
    """results: per-core {"yout": [NPAIR, 128, 1024] f16} -> [B, OH, OW, F] f32."""
    ys = []
    for r in results:
        yd = np.asarray(r["yout"]).astype(np.float32)
        y6 = yd.reshape(4, 4, 32, 2, 16, 32)   # [pair, bank, batch, ohp, posin, f]
        y6 = y6.transpose(2, 0, 3, 1, 4, 5)     # [batch, pair, ohp, bank, posin, f]
        ys.append(y6.reshape(32, 8, 64, 32))
    y = np.concatenate(ys, axis=1)              # [B, 8*ncores, OWP, F]
    return np.ascontiguousarray(y[:, :min(OH, y.shape[1]), :OW, :])


def run(inputs, trace=False, trace_cores=None):
    """Build/compile/run on 8 cores. Returns (y, BassKernelResults)."""
    from concourse.bass_utils import run_bass_kernel_spmd

    nc = _get_program()
    in_maps = _prep_inputs(**inputs)
    res = run_bass_kernel_spmd(
        nc,
        in_maps,
        core_ids=list(range(8)),
        trace=trace,
        **({"trace_cores": trace_cores} if trace_cores is not None else {}),
    )
    return _assemble_output(res.results), res


def kernel(x, gamma, beta, moving_mean, moving_var, weight):
    y, _ = run(
        dict(x=x, gamma=gamma, beta=beta, moving_mean=moving_mean,
             moving_var=moving_var, weight=weight)
    )
    return y


# revision 45
# speedup vs baseline: 1.1180x; 1.0263x over previous
"""Trainium2 Bass kernel for nn_BlockRF (BatchNorm -> LocallyConnected2D 3x3 valid -> ReLU).

Shapes (hardcoded per the problem spec):
  x:      [B=32, H=64, W=64, C=32]  f32
  gamma/beta/moving_mean/moving_var: [C=32] f32
  weight: [OH*OW=3844, KH*KW*C=288, F=32] f32
  out:    [B=32, OH=62, OW=62, F=32] f32

v2 design (memory regime: the weight tensor dominates traffic):
  - Shard over output rows: 8 rows/core on 8 cores (OH padded 62->64),
    processed as 4 row PAIRS per core.
  - Weights shipped as fp8 E3M4 (x32 exponent centering, folded back into the
    BN affine on the x side) -> halves the dominant HBM stream. Mixed-dtype
    matmul: fp16 stationary x fp8 moving (empirical rel-err ~1.3e-2 vs the
    2e-2 gate).
  - One x stationary [128,32] (rows oh..oh+3 x 32ch) serves BOTH rows of a
    pair: even row matmuls take K=96 (rows 0..95), odd row K=128 against a
    weight tile whose rows 0..31 are zero (host-padded into the odd DMA).
  - PE col groups: group b = tile_position (0,32b) owns position bank b
    (16 positions); each output row's PSUM is a single [128,512] bank
    (4 groups x disjoint 32-partition blocks), 8 banks in flight.
  - Weight streams are split into wave-range chunks (TSPLITS): matmuls of a
    chunk start as soon as its transfer lands, so the PE tracks the stream
    and never idles long enough for the HAM clock gate to re-throttle. The
    chunks alternate between the sync and scalar HWDGE rings (balanced per
    pair); x + y ride the gpsimd SWDGE ring (y of the last pair takes the
    sync ring for its lower tail latency). 3-deep weight / 4-deep x
    buffering keeps every DMA queue busy without write-after-read stalls.
  - memset+accumulate PSUM discipline, all BNs + memsets hoisted ahead of
    any evacuation on the strict-FIFO Vector queue; ReLU evacuation as one
    [128,512] Vector op per row (no Scalar activations -> no ACT table load
    on the critical path); one [128,2KB] output DMA per pair.

Host side pads/transposes/casts (layout prep + sharding) and folds the BN
moving stats into two per-channel affine coefficients (standard inference
BN fold); the per-activation normalize, convolution and ReLU - all the
O(B*H*W*C) model arithmetic - run on device.
"""

import numpy as np
import ml_dtypes

B, H, W, C, F = 32, 64, 64, 32, 32
KH = KW = 3
OH = OW = 62
OHP = OWP = 64
RPC = 8          # output rows per core
NPAIR = RPC // 2
XFREE = W * B    # 2048
EPS = 1e-3
WSCALE = 32.0    # weight exponent centering for fp8 E3M4
NGRP = 4         # PE col groups == position banks of 16
TSPLITS = [0, 7, 13, 18]  # wave ranges of the weight-stream chunks


def _build_sched():
    """Static MM schedule. Group b (PE col-group b) owns positions
    16b..16b+15 (its own single-bank PSUM region at partitions 32b). Its
    w-list covers w=16b+t: t=0..15 main (position subrange clipped to the
    bank) and t=16,17 boundary work for positions 14,15 of its bank coming
    from w in the next group's range.

    The schedule is cut into len(TSPLITS)-1 wave-range chunks; each chunk's
    slots are packed contiguously so its weights ship as one transfer and
    its matmuls start as soon as that transfer lands (and the PE never
    idles long enough for the HAM clock gate to re-throttle).

    Returns a list of (issue, nslot) per chunk; issue entries are
    (b, w, plo, phi, slot) with plo/phi bank-local, slot chunk-local."""
    pergroup = {}
    for b in range(NGRP):
        tmax = 17 if b < NGRP - 1 else 15
        for t in range(tmax + 1):
            w = 16 * b + t
            if t == 0:
                lo, hi = 16 * b, 16 * b
            elif t == 1:
                lo, hi = 16 * b, 16 * b + 1
            elif t <= 15:
                lo, hi = 16 * b + t - 2, 16 * b + t
            elif t == 16:
                lo, hi = 16 * b + 14, 16 * b + 15
            else:
                lo, hi = 16 * b + 15, 16 * b + 15
            pergroup[(b, t)] = (w, lo, hi)
    chunks = []
    for ci in range(len(TSPLITS) - 1):
        issue = []
        off = 0
        for t in range(TSPLITS[ci], TSPLITS[ci + 1]):
            for b in range(NGRP):
                if (b, t) not in pergroup:
                    continue
                w, lo, hi = pergroup[(b, t)]
                issue.append((b, w, lo - 16 * b, hi - 16 * b, off))
                off += hi - lo + 1
        chunks.append((issue, off))
    return chunks


CHUNKS = _build_sched()
NCHUNK = len(CHUNKS)
SLOTF_K = [n * F for (_, n) in CHUNKS]

_CACHE = {}


def _build_program():
    import concourse.mybir as mybir
    import concourse.tile as tile
    from concourse import bacc
    from contextlib import ExitStack

    f16 = mybir.dt.float16
    f32 = mybir.dt.float32
    f8 = mybir.dt.float8e3

    nc = bacc.Bacc("TRN2", target_bir_lowering=False, debug=False, num_devices=8)

    xin = nc.dram_tensor("xin", [NPAIR, 128, XFREE], f16, kind="ExternalInput").ap()
    weink = [nc.dram_tensor(f"wein{k}", [NPAIR, 96, SLOTF_K[k]], f8,
                            kind="ExternalInput").ap() for k in range(NCHUNK)]
    woink = [nc.dram_tensor(f"woin{k}", [NPAIR, 128, SLOTF_K[k]], f8,
                            kind="ExternalInput").ap() for k in range(NCHUNK)]
    pin = nc.dram_tensor("pin", [128, 2], f32, kind="ExternalInput").ap()
    yout = nc.dram_tensor("yout", [NPAIR, 128, 1024], f16, kind="ExternalOutput").ap()

    with ExitStack() as ctx:
        tc = ctx.enter_context(tile.TileContext(nc))
        singles = ctx.enter_context(tc.tile_pool(name="singles", bufs=1))
        xpool = ctx.enter_context(tc.tile_pool(name="xpool", bufs=4))
        xnpool = ctx.enter_context(tc.tile_pool(name="xnpool", bufs=4))
        wepools = [ctx.enter_context(tc.tile_pool(name=f"wep{k}", bufs=3))
                   for k in range(NCHUNK)]
        wopools = [ctx.enter_context(tc.tile_pool(name=f"wop{k}", bufs=3))
                   for k in range(NCHUNK)]
        ypool = ctx.enter_context(tc.tile_pool(name="ypool", bufs=2))
        pspool = ctx.enter_context(tc.tile_pool(name="pspool", bufs=8, space="PSUM"))

        xts = []

        def fetch_x(p):
            xt = xpool.tile([128, XFREE], f16, name="xt", tag="xt")
            nc.gpsimd.dma_start(out=xt, in_=xin[p])
            xts.append(xt)

        # BN coefficients (1KB) lead the sync ring; x leads the gpsimd ring;
        # the two HWDGE rings otherwise carry the weight streams, balanced
        # per pair (even-A + odd-B on sync, odd-A + even-B on scalar)
        par = singles.tile([128, 2], f32)
        nc.sync.dma_start(out=par, in_=pin)
        for p in range(NPAIR):
            fetch_x(p)

        wek = {}
        wok = {}

        def fetch_weights(p):
            for k in range(NCHUNK):
                we = wepools[k].tile([96, SLOTF_K[k]], f8, name=f"we{k}",
                                     tag=f"we{k}")
                wo = wopools[k].tile([128, SLOTF_K[k]], f8, name=f"wo{k}",
                                     tag=f"wo{k}")
                if k % 2 == 0:
                    nc.sync.dma_start(out=we, in_=weink[k][p])
                    nc.scalar.dma_start(out=wo, in_=woink[k][p])
                else:
                    nc.scalar.dma_start(out=we, in_=weink[k][p])
                    nc.sync.dma_start(out=wo, in_=woink[k][p])
                wek[(p, k)] = we
                wok[(p, k)] = wo

        fetch_weights(0)

        def psum_alloc():
            ps = pspool.tile([128, 512], f32, name="ps", tag="ps")
            nc.vector.memset(ps, 0.0)
            return ps

        # BN affine coefficients are folded on the host (standard inference
        # BN fold): par[:,0] = gamma/sqrt(var+eps)/WSCALE, par[:,1] = (beta -
        # mean*gamma/sqrt(var+eps))/WSCALE. The per-activation normalize
        # itself runs here on the Vector engine.
        def bn(p):
            xn = xnpool.tile([128, XFREE], f16, name="xn", tag="xn")
            nc.vector.tensor_scalar(
                xn, xts[p], par[:, 0:1], par[:, 1:2],
                op0=mybir.AluOpType.mult, op1=mybir.AluOpType.add,
            )
            return xn

        # all BNs + PSUM memsets up front on the vector queue (8 PSUM banks
        # = exactly 4 pairs, so nothing recycles and nothing queues behind
        # an evacuation)
        xns, pss = {}, {}
        for p in range(NPAIR):
            xns[p] = bn(p)
            pss[p] = (psum_alloc(), psum_alloc())

        def emit_half(issue, xn, pse, pso, wet, wot):
            for (b, w, plo, phi, s) in issue:
                n = phi - plo + 1
                ocol = slice(plo * F, (phi + 1) * F)
                wcol = slice(s * F, (s + n) * F)
                xcol = slice(w * B, (w + 1) * B)
                nc.tensor.matmul(
                    pso[32 * b:32 * b + 32, ocol],
                    xn[0:128, xcol],
                    wot[0:128, wcol],
                    start=False, stop=True,
                    skip_group_check=True,
                    tile_position=(0, 32 * b),
                )
                nc.tensor.matmul(
                    pse[32 * b:32 * b + 32, ocol],
                    xn[0:96, xcol],
                    wet[0:96, wcol],
                    start=False, stop=True,
                    skip_group_check=True,
                    tile_position=(0, 32 * b),
                )

        fetch_weights(1)
        fetch_weights(2)

        for p in range(NPAIR):
            if p + 3 < NPAIR:
                fetch_weights(p + 3)

            xn = xns[p]
            pse, pso = pss[p]
            for k in range(NCHUNK):
                emit_half(CHUNKS[k][0], xn, pse, pso,
                          wek[(p, k)], wok[(p, k)])

            yb = ypool.tile([128, 1024], f16, name="yb", tag="yb")
            nc.vector.tensor_scalar_max(yb[:, 0:512], pse, 0.0)
            nc.vector.tensor_scalar_max(yb[:, 512:1024], pso, 0.0)
            # y0-2 ride the (by then idle) gpsimd ring; y3 takes the sync
            # HWDGE ring for its lower dispatch latency on the kernel tail
            if p == NPAIR - 1:
                nc.sync.dma_start(out=yout[p], in_=yb)
            else:
                nc.gpsimd.dma_start(out=yout[p], in_=yb)

    nc.compile()
    return nc


def _get_program():
    if "nc" not in _CACHE:
        _CACHE["nc"] = _build_program()
    return _CACHE["nc"]


def _prep_inputs(x, gamma, beta, moving_mean, moving_var, weight):
    """Host-side shard/layout/cast prep. Returns per-core in_maps."""
    x = np.asarray(x, dtype=np.float32)
    weight = np.asarray(weight, dtype=np.float32)

    # x: [B,H,W,C] -> pad H to 66 -> [h, c, w, b] fp16
    xpad = np.zeros((B, H + 2, W, C), np.float32)
    xpad[:, :H] = x
    xt_all = np.ascontiguousarray(xpad.transpose(1, 3, 2, 0)).astype(np.float16)

    # weights -> wave-major slot layout per stream chunk, fp8 E3M4, x WSCALE
    w6 = weight.reshape(OH, OW, KH, KW, C, F) * WSCALE
    wpad = np.zeros((OHP, OWP, KH, KW, C, F), np.float32)
    wpad[:OH, :OW] = w6
    wks = []
    for (issue, nslot) in CHUNKS:
        dst = np.zeros((OHP, 96, nslot, F), np.float32)
        for (b, w, plo, phi, s) in issue:
            for k, pl in enumerate(range(plo, phi + 1)):
                pos = 16 * b + pl
                j = w - pos
                dst[:, :, s + k, :] = wpad[:, pos, :, j, :, :].reshape(
                    OHP, 96, F)
        wks.append(dst.astype(ml_dtypes.float8_e3m4))

    inv = (gamma / np.sqrt(moving_var + EPS)).astype(np.float32)
    acoef = inv / WSCALE
    bcoef = (beta - moving_mean * inv) / WSCALE
    p128 = np.tile(
        np.stack([acoef, bcoef], axis=1).astype(np.float32), (4, 1)
    )  # [128, 2]

    in_maps = []
    for k in range(8):
        R = k * RPC
        xc = np.stack(
            [xt_all[R + 2 * p: R + 2 * p + 4].reshape(128, XFREE)
             for p in range(NPAIR)]
        )  # [NPAIR, 128, 2048]
        im = {"xin": xc, "pin": p128}
        for ci, wk in enumerate(wks):
            sf = SLOTF_K[ci]
            im[f"wein{ci}"] = np.ascontiguousarray(
                wk[R + 0: R + RPC: 2]).reshape(NPAIR, 96, sf)
            # odd rows: zero band at partitions 0..31 (K=128 matmul)
            wo = np.zeros((NPAIR, 128, sf), ml_dtypes.float8_e3m4)
            wo[:, 32:] = wk[R + 1: R + RPC: 2].reshape(NPAIR, 96, sf)
            im[f"woin{ci}"] = wo
        in_maps.append(im)
    return in_maps


def _assemble_output(results):
    """results: per-core {"yout": [NPAIR, 128, 1024] f16} -> [B, OH, OW, F] f32."""
    ys = []
    for r in results:
        yd = np.asarray(r["yout"]).astype(np.float32)
        y6 = yd.reshape(4, 4, 32, 2, 16, 32)   # [pair, bank, batch, ohp, posin, f]
        y6 = y6.transpose(2, 0, 3, 1, 4, 5)     # [batch, pair, ohp, bank, posin, f]
        ys.append(y6.reshape(32, 8, 64, 32))
    y = np.concatenate(ys, axis=1)              # [B, 8*ncores, OWP, F]
    return np.ascontiguousarray(y[:, :min(OH, y.shape[1]), :OW, :])


def run(inputs, trace=False, trace_cores=None):
    """Build/compile/run on 8 cores. Returns (y, BassKernelResults)."""
    from concourse.bass_utils import run_bass_kernel_spmd

    nc = _get_program()
    in_maps = _prep_inputs(**inputs)
    res = run_bass_kernel_spmd(
        nc,
        in_maps,
        core_ids=list(range(8)),
        trace=trace,
        **({"trace_cores": trace_cores} if trace_cores is not None else {}),
    )
    return _assemble_output(res.results), res


def kernel(x, gamma, beta, moving_mean, moving_var, weight):
    y, _ = run(
        dict(x=x, gamma=gamma, beta=beta, moving_mean=moving_mean,
             moving_var=moving_var, weight=weight)
    )
    return y
